# revision 11
# baseline (speedup 1.0000x reference)
"""Binary-weight dense layer on 8 TRN2 NeuronCores.

Computes out = x @ sign(W) + b for x:[8192,4096] f32, W:[4096,4096] f32,
b:[4096] f32, sharded row-wise over x (tensor-parallel over the batch dim:
each core computes a [1024, 4096] slice of the output; no collectives).

Per-core kernel strategy (single bf16 pass, n-slice-paired matmuls):
  - x is cast to bf16 once; sign(W) is exact in bf16, so the only error is
    the bf16 rounding of x (~1.7e-3 output rel err, well under the 2e-2
    gate). One bf16 matmul pass is half the PE time of an fp32-recovery
    hi/lo scheme, and fp8 double-pumping cannot beat it at this accuracy.
  - Prep: x is staged f32->bf16 with casting SWDGE DMAs (DRAM->DRAM, no
    SBUF round trip), then xbar-transposed per k-tile into an SBUF-resident
    [K=128, M=1024] lhsT layout (2-byte path), chunked so transposes and
    matmuls start while later chunks still stage.
  - Main loop processes n-slice PAIRS (1024 W columns): the signed W tiles
    for a pair are kept SBUF-resident across both m-halves so W still
    streams from HBM exactly once, and each stationary xt strip feeds TWO
    matmuls (one per n-slice of the pair), halving the per-matmul
    weight-load overhead on the PE. PSUM: 4 m-tiles x 2 n-slices = 8 banks
    accumulate over all 32 k-tiles; eviction is a DVE bias-add.
"""

import sys

if "/opt/trn_rl_repo" not in sys.path:
    sys.path.insert(0, "/opt/trn_rl_repo")

import numpy as np

import concourse.bass as bass
import concourse.mybir as mybir
import concourse.tile as tile
from concourse import bacc
from concourse.bass_utils import run_bass_kernel_spmd

N_CORES = 8
P = 128

B, N_IN, N_UNITS = 8192, 4096, 4096
M_SH = B // N_CORES  # 1024 rows of x per core

F32 = mybir.dt.float32
BF16 = mybir.dt.bfloat16


def build_module(m_sh=M_SH, k_dim=N_IN, n_dim=N_UNITS, reps=1, timing=False):
    """Build + compile the per-core Bass module (same program on all cores).

    reps>1 wraps the whole pipeline in a hardware For_i loop and timing=True
    swaps the big output for an internal DRAM tensor plus a tiny sink output;
    both are used only for wall-clock timing calibration (the marginal cost of
    an extra rep is the kernel's HW exec time, free of host/tunnel overhead)."""
    nc = bacc.Bacc("TRN2", target_bir_lowering=False, debug=False)

    x_in = nc.dram_tensor("x", [m_sh, k_dim], F32, kind="ExternalInput")
    w_in = nc.dram_tensor("W", [k_dim, n_dim], F32, kind="ExternalInput")
    b_in = nc.dram_tensor("b", [n_dim], F32, kind="ExternalInput")
    if timing:
        out = nc.dram_tensor("out_scratch", [m_sh, n_dim], F32)
        sink = nc.dram_tensor("out", [P, 512], F32, kind="ExternalOutput")
    else:
        out = nc.dram_tensor("out", [m_sh, n_dim], F32, kind="ExternalOutput")

    NT = 512  # psum free dim (one bank of fp32)
    KT = P  # contraction tile
    m_tiles = m_sh // P
    k_tiles = k_dim // KT
    PREP_C = min(512, k_dim)  # prep chunk of the k axis (staging granularity)
    prep_chunks = k_dim // PREP_C
    NSP = n_dim // (2 * NT)  # pairs of 512-col n-slices
    MH = m_tiles // 4  # m-halves (4 m-tiles per psum group)

    b_2d = b_in.ap().rearrange("(a n) -> a n", a=1)

    import contextlib

    with tile.TileContext(nc) as tc:
        with (
            tc.For_i(0, reps, 1) if reps > 1 else contextlib.nullcontext(),
            tc.tile_pool(name="dram", bufs=1, space="DRAM") as dram,
            tc.tile_pool(name="xt", bufs=1) as xt_pool,
            tc.tile_pool(name="bias", bufs=4) as bias_pool,
            tc.tile_pool(name="wf", bufs=4) as wf_pool,
            tc.tile_pool(name="wqr", bufs=34) as wq_pool,
            tc.tile_pool(name="psum", bufs=8, space="PSUM") as psum_pool,
            tc.tile_pool(name="osb", bufs=4) as out_pool,
        ):
            # bf16 staging copy of x in DRAM (written by casting SWDGE DMAs).
            x_bf16_dram = dram.tile([m_sh, k_dim], BF16)

            # SBUF-resident transposed activations: column block kt holds
            # [K=128, M=m_sh] for contraction tile kt.
            xt = xt_pool.tile([P, k_tiles * m_sh], BF16)

            # ---- Stage 1: cast-stage x to bf16 in DRAM with row-major
            # per-m-tile SWDGE ops (contiguous 16KB-row descriptors, cheap
            # Q7 emission). The xbar transposes into the SBUF lhsT layout
            # are emitted inside the nsp==0 body AFTER its W loads, so the
            # SP ring delivers pair-0's W while staging completes and the
            # transposes then deliver k-tiles just ahead of PE consumption.
            for mt in range(m_tiles):
                ms = slice(mt * P, (mt + 1) * P)
                nc.gpsimd.dma_start(x_bf16_dram[ms, :], x_in[ms, :])

            def emit_transposes():
                for kt in range(k_tiles):
                    ks = slice(kt * KT, (kt + 1) * KT)
                    os_ = slice(kt * m_sh, (kt + 1) * m_sh)
                    nc.sync.dma_start_transpose(xt[:, os_], x_bf16_dram[:, ks])

            # ---- Stage 2: main matmul loop over n-slice pairs ----
            osb = None
            for nsp in range(NSP):
                csl = slice(nsp * 2 * NT, (nsp + 1) * 2 * NT)
                bts = []
                for j in range(2):
                    bt = bias_pool.tile([P, NT], F32, name=f"bt_{nsp}_{j}", tag="bt")
                    nss = slice((2 * nsp + j) * NT, (2 * nsp + j + 1) * NT)
                    nc.sync.dma_start(bt[:], b_2d[:, nss].broadcast_to([P, NT]))
                    bts.append(bt)
                wqs = []
                for kt in range(k_tiles):
                    wf = wf_pool.tile([P, 2 * NT], F32, name=f"wf_{nsp}_{kt}", tag="wf")
                    nc.sync.dma_start(wf[:], w_in[kt * KT : (kt + 1) * KT, csl])
                    wq = wq_pool.tile([P, 2 * NT], BF16, name=f"wq_{nsp}_{kt}", tag="wq")
                    nc.scalar.sign(wq[:], wf[:])
                    wqs.append(wq)
                if nsp == 0:
                    emit_transposes()
                for mh in range(MH):
                    psums = [
                        psum_pool.tile([P, NT], F32, name=f"ps_{nsp}_{mh}_{i}", tag="ps")
                        for i in range(8)
                    ]
                    for kt in range(k_tiles):
                        for mt in range(4):
                            xo = kt * m_sh + (mh * 4 + mt) * P
                            for j in range(2):
                                nc.tensor.matmul(
                                    psums[2 * mt + j][:],
                                    xt[:, xo : xo + P],
                                    wqs[kt][:, j * NT : (j + 1) * NT],
                                    start=(kt == 0),
                                    stop=(kt == k_tiles - 1),
                                )
                    for mt in range(4):
                        M = mh * 4 + mt
                        for j in range(2):
                            nss = slice((2 * nsp + j) * NT, (2 * nsp + j + 1) * NT)
                            osb = out_pool.tile(
                                [P, NT], F32, name=f"osb_{nsp}_{mh}_{mt}_{j}", tag="osb"
                            )
                            nc.vector.tensor_add(osb[:], psums[2 * mt + j][:], bts[j][:])
                            nc.sync.dma_start(out[M * P : (M + 1) * P, nss], osb[:])
            if timing:
                nc.sync.dma_start(sink[:], osb[:])

    nc.compile()
    return nc


_NC_CACHE = {}


def _get_module(m_sh=M_SH, k_dim=N_IN, n_dim=N_UNITS):
    key = (m_sh, k_dim, n_dim)
    if key not in _NC_CACHE:
        _NC_CACHE[key] = build_module(m_sh, k_dim, n_dim)
    return _NC_CACHE[key]


def kernel(x: np.ndarray, W: np.ndarray, b: np.ndarray) -> np.ndarray:
    x = np.ascontiguousarray(np.asarray(x, dtype=np.float32))
    W = np.ascontiguousarray(np.asarray(W, dtype=np.float32))
    b = np.ascontiguousarray(np.asarray(b, dtype=np.float32))
    assert x.shape == (B, N_IN) and W.shape == (N_IN, N_UNITS) and b.shape == (N_UNITS,)

    nc = _get_module()
    in_maps = [
        {"x": x[i * M_SH : (i + 1) * M_SH], "W": W, "b": b} for i in range(N_CORES)
    ]
    res = run_bass_kernel_spmd(nc, in_maps, core_ids=list(range(N_CORES)))
    return np.concatenate(
        [res.results[i]["out"] for i in range(N_CORES)], axis=0
    ).astype(np.float32)


# revision 12
# speedup vs baseline: 1.0511x; 1.0511x over previous
"""Binary-weight dense layer on 8 TRN2 NeuronCores.

Computes out = x @ sign(W) + b for x:[8192,4096] f32, W:[4096,4096] f32,
b:[4096] f32, sharded row-wise over x (tensor-parallel over the batch dim:
each core computes a [1024, 4096] slice of the output; no collectives).

Per-core kernel strategy (single bf16 pass, n-slice-paired matmuls):
  - x is cast to bf16 once; sign(W) is exact in bf16, so the only error is
    the bf16 rounding of x (~1.7e-3 output rel err, well under the 2e-2
    gate). One bf16 matmul pass is half the PE time of an fp32-recovery
    hi/lo scheme, and fp8 double-pumping cannot beat it at this accuracy.
  - Prep: x is staged f32->bf16 with casting SWDGE DMAs (DRAM->DRAM, no
    SBUF round trip), then xbar-transposed per k-tile into an SBUF-resident
    [K=128, M=1024] lhsT layout (2-byte path), chunked so transposes and
    matmuls start while later chunks still stage.
  - Main loop processes n-slice PAIRS (1024 W columns): the signed W tiles
    for a pair are kept SBUF-resident across both m-halves so W still
    streams from HBM exactly once, and each stationary xt strip feeds TWO
    matmuls (one per n-slice of the pair), halving the per-matmul
    weight-load overhead on the PE. PSUM: 4 m-tiles x 2 n-slices = 8 banks
    accumulate over all 32 k-tiles; eviction is a DVE bias-add.
"""

import sys

if "/opt/trn_rl_repo" not in sys.path:
    sys.path.insert(0, "/opt/trn_rl_repo")

import numpy as np

import concourse.bass as bass
import concourse.mybir as mybir
import concourse.tile as tile
from concourse import bacc
from concourse.bass_utils import run_bass_kernel_spmd

N_CORES = 8
P = 128

B, N_IN, N_UNITS = 8192, 4096, 4096
M_SH = B // N_CORES  # 1024 rows of x per core

F32 = mybir.dt.float32
BF16 = mybir.dt.bfloat16


def build_module(m_sh=M_SH, k_dim=N_IN, n_dim=N_UNITS, reps=1, timing=False):
    """Build + compile the per-core Bass module (same program on all cores).

    reps>1 wraps the whole pipeline in a hardware For_i loop and timing=True
    swaps the big output for an internal DRAM tensor plus a tiny sink output;
    both are used only for wall-clock timing calibration (the marginal cost of
    an extra rep is the kernel's HW exec time, free of host/tunnel overhead)."""
    nc = bacc.Bacc("TRN2", target_bir_lowering=False, debug=False)

    x_in = nc.dram_tensor("x", [m_sh, k_dim], F32, kind="ExternalInput")
    w_in = nc.dram_tensor("W", [k_dim, n_dim], F32, kind="ExternalInput")
    b_in = nc.dram_tensor("b", [n_dim], F32, kind="ExternalInput")
    if timing:
        out = nc.dram_tensor("out_scratch", [m_sh, n_dim], F32)
        sink = nc.dram_tensor("out", [P, 512], F32, kind="ExternalOutput")
    else:
        out = nc.dram_tensor("out", [m_sh, n_dim], F32, kind="ExternalOutput")

    NT = 512  # psum free dim (one bank of fp32)
    KT = P  # contraction tile
    m_tiles = m_sh // P
    k_tiles = k_dim // KT
    PREP_C = min(512, k_dim)  # prep chunk of the k axis (staging granularity)
    prep_chunks = k_dim // PREP_C
    NSP = n_dim // (2 * NT)  # pairs of 512-col n-slices
    MH = m_tiles // 4  # m-halves (4 m-tiles per psum group)

    b_2d = b_in.ap().rearrange("(a n) -> a n", a=1)

    import contextlib

    with tile.TileContext(nc) as tc:
        with (
            tc.For_i(0, reps, 1) if reps > 1 else contextlib.nullcontext(),
            tc.tile_pool(name="dram", bufs=1, space="DRAM") as dram,
            tc.tile_pool(name="xt", bufs=1) as xt_pool,
            tc.tile_pool(name="bias", bufs=4) as bias_pool,
            tc.tile_pool(name="wf", bufs=6) as wf_pool,
            tc.tile_pool(name="wqr", bufs=36) as wq_pool,
            tc.tile_pool(name="psum", bufs=8, space="PSUM") as psum_pool,
            tc.tile_pool(name="osb", bufs=4) as out_pool,
        ):
            # bf16 staging copy of x in DRAM (written by casting SWDGE DMAs).
            x_bf16_dram = dram.tile([m_sh, k_dim], BF16)

            # SBUF-resident transposed activations: column block kt holds
            # [K=128, M=m_sh] for contraction tile kt.
            xt = xt_pool.tile([P, k_tiles * m_sh], BF16)

            # ---- Stage 1: cast-stage x to bf16 in DRAM with row-major
            # per-m-tile SWDGE ops (contiguous 16KB-row descriptors, cheap
            # Q7 emission). The xbar transposes into the SBUF lhsT layout
            # are emitted inside the nsp==0 body AFTER its W loads, so the
            # SP ring delivers pair-0's W while staging completes and the
            # transposes then deliver k-tiles just ahead of PE consumption.
            for mt in range(m_tiles):
                ms = slice(mt * P, (mt + 1) * P)
                nc.gpsimd.dma_start(x_bf16_dram[ms, :], x_in[ms, :])

            def emit_transposes():
                for kt in range(k_tiles):
                    ks = slice(kt * KT, (kt + 1) * KT)
                    os_ = slice(kt * m_sh, (kt + 1) * m_sh)
                    nc.sync.dma_start_transpose(xt[:, os_], x_bf16_dram[:, ks])

            # ---- Stage 2: main matmul loop over n-slice pairs ----
            osb = None
            for nsp in range(NSP):
                csl = slice(nsp * 2 * NT, (nsp + 1) * 2 * NT)
                bts = []
                for j in range(2):
                    bt = bias_pool.tile([P, NT], F32, name=f"bt_{nsp}_{j}", tag="bt")
                    nss = slice((2 * nsp + j) * NT, (2 * nsp + j + 1) * NT)
                    nc.sync.dma_start(bt[:], b_2d[:, nss].broadcast_to([P, NT]))
                    bts.append(bt)
                wqs = []
                for kt in range(k_tiles):
                    wf = wf_pool.tile([P, 2 * NT], F32, name=f"wf_{nsp}_{kt}", tag="wf")
                    nc.sync.dma_start(wf[:], w_in[kt * KT : (kt + 1) * KT, csl])
                    wq = wq_pool.tile([P, 2 * NT], BF16, name=f"wq_{nsp}_{kt}", tag="wq")
                    nc.scalar.sign(wq[:], wf[:])
                    wqs.append(wq)
                if nsp == 0:
                    emit_transposes()
                for mh in range(MH):
                    psums = [
                        psum_pool.tile([P, NT], F32, name=f"ps_{nsp}_{mh}_{i}", tag="ps")
                        for i in range(8)
                    ]
                    for kt in range(k_tiles):
                        for mt in range(4):
                            xo = kt * m_sh + (mh * 4 + mt) * P
                            for j in range(2):
                                nc.tensor.matmul(
                                    psums[2 * mt + j][:],
                                    xt[:, xo : xo + P],
                                    wqs[kt][:, j * NT : (j + 1) * NT],
                                    start=(kt == 0),
                                    stop=(kt == k_tiles - 1),
                                )
                    for mt in range(4):
                        M = mh * 4 + mt
                        for j in range(2):
                            nss = slice((2 * nsp + j) * NT, (2 * nsp + j + 1) * NT)
                            osb = out_pool.tile(
                                [P, NT], F32, name=f"osb_{nsp}_{mh}_{mt}_{j}", tag="osb"
                            )
                            nc.vector.tensor_add(osb[:], psums[2 * mt + j][:], bts[j][:])
                            nc.sync.dma_start(out[M * P : (M + 1) * P, nss], osb[:])
            if timing:
                nc.sync.dma_start(sink[:], osb[:])

    nc.compile()
    return nc


_NC_CACHE = {}


def _get_module(m_sh=M_SH, k_dim=N_IN, n_dim=N_UNITS):
    key = (m_sh, k_dim, n_dim)
    if key not in _NC_CACHE:
        _NC_CACHE[key] = build_module(m_sh, k_dim, n_dim)
    return _NC_CACHE[key]


def kernel(x: np.ndarray, W: np.ndarray, b: np.ndarray) -> np.ndarray:
    x = np.ascontiguousarray(np.asarray(x, dtype=np.float32))
    W = np.ascontiguousarray(np.asarray(W, dtype=np.float32))
    b = np.ascontiguousarray(np.asarray(b, dtype=np.float32))
    assert x.shape == (B, N_IN) and W.shape == (N_IN, N_UNITS) and b.shape == (N_UNITS,)

    nc = _get_module()
    in_maps = [
        {"x": x[i * M_SH : (i + 1) * M_SH], "W": W, "b": b} for i in range(N_CORES)
    ]
    res = run_bass_kernel_spmd(nc, in_maps, core_ids=list(range(N_CORES)))
    return np.concatenate(
        [res.results[i]["out"] for i in range(N_CORES)], axis=0
    ).astype(np.float32)
